# revision 17
# baseline (speedup 1.0000x reference)
"""Trainium2 Bass kernel for CachedMixtralAttention (sliding-window GQA attention).

Strategy (8 NeuronCores, tensor-parallel over KV-head groups):
  - Core i handles KV head i and its 4 query heads (GQA group). Wq/Wk/Wv are
    sliced on the head axis, Wo on the input-head axis. Each core computes a
    partial output [S, HID] in bf16; the host sums the 8 partials in fp32.
  - Single software-pipelined PE stream: while the QKV projection of chunk c
    streams on the PE, the attention of chunk c-1 and the output projection of
    chunk c-2 are interleaved into the same PE queue, so the tensor engine
    never waits on the exp/mask/normalize chains (those run concurrently on
    the scalar/vector/gpsimd engines).
  - Softmax denominator via a ones-vector matmul on the PE ([1,512] PSUM
    accumulate) + reciprocal_approx_fast + a broadcast matmul, instead of
    gpsimd partition reduces and full-width reciprocals.
  - PSUM budget (8 banks): 3 proj accumulators (two groups of 3 per chunk),
    1 score, 1 attention accumulator, 1 denominator, 2 outproj/bcast/transpose.
  - All element-wise math in bf16 (2x DVE rate); PSUM accumulation fp32.
"""

from contextlib import ExitStack

import ml_dtypes
import numpy as np

S = 2048
HID = 4096
NUM_Q_HEADS = 32
NUM_KV_HEADS = 8
D = 128                      # head dim
NCORES = 8
HQ = NUM_Q_HEADS // NUM_KV_HEADS  # q heads per core (GQA group size)
QC = 512                     # query chunk (matmul moving free dim)
HB = 4                       # hid tiles per hT DMA batch
MAX_WAVELENGTH = 10000.0
INV_NORM = 1.0 / np.sqrt(D)

BF16 = ml_dtypes.bfloat16


def _rope_tables(s):
    """cos/sin tables in T layout [128, s], sign folded into sin."""
    pos = np.arange(s, dtype=np.float32)
    invf = 1.0 / (MAX_WAVELENGTH ** (np.arange(0, D, 2, dtype=np.float32) / D))
    freq = invf[:, None] * pos[None, :]              # [64, s]
    cosT = np.concatenate([np.cos(freq), np.cos(freq)], axis=0)   # [128, s]
    sinT = np.concatenate([-np.sin(freq), np.sin(freq)], axis=0)  # [128, s]
    return cosT.astype(BF16), sinT.astype(BF16)


def _classify_mask(mask2d, s):
    """Classify [128k x QCq] blocks of the mask: skip / full / partial.

    Returns (blocks, mask_tiles): blocks[c] is a list of (g, mask_id) with
    g the global k-tile index and mask_id None for full blocks; mask_tiles
    is [n, 128, QC] float32 of the partial blocks (n >= 1, padded).
    """
    mT = np.ascontiguousarray(mask2d.T)  # [k, q]
    n_chunks = s // QC
    n_ktiles = s // 128
    blocks = []
    tiles = []
    tile_ids = {}
    for c in range(n_chunks):
        lst = []
        for g in range(n_ktiles):
            blk = mT[g * 128:(g + 1) * 128, c * QC:(c + 1) * QC]
            if not blk.any():
                continue
            if blk.all():
                lst.append((g, None))
            else:
                key = blk.tobytes()
                if key not in tile_ids:
                    tile_ids[key] = len(tiles)
                    tiles.append(blk.astype(np.float32))
                lst.append((g, tile_ids[key]))
        assert lst, f"query chunk {c} attends to nothing"
        blocks.append(lst)
    if not tiles:
        tiles.append(np.zeros((128, QC), np.float32))
    return blocks, np.stack(tiles)


def _build_program(s, hid, blocks, n_mask):
    """Emit the Bass/Tile program. Same program runs SPMD on all 8 cores."""
    import concourse.bacc as bacc
    import concourse.mybir as mybir
    import concourse.tile as tile

    dt = mybir.dt
    HT = hid // 128          # hidden contraction tiles (32)
    C = s // QC              # query chunks (4)
    LOOK = 3                 # score lookahead inside the attention stream

    nc = bacc.Bacc("TRN2", target_bir_lowering=False, debug=False,
                   num_devices=NCORES)

    hT_d = nc.declare_dram_parameter("hT", [128, HT * s], dt.bfloat16, isOutput=False)
    wq_d = nc.declare_dram_parameter("wq", [128, hid * HQ], dt.bfloat16, isOutput=False)
    wk_d = nc.declare_dram_parameter("wk", [128, hid], dt.bfloat16, isOutput=False)
    wv_d = nc.declare_dram_parameter("wv", [128, hid], dt.bfloat16, isOutput=False)
    wo_d = nc.declare_dram_parameter("wo", [128, HQ * hid], dt.bfloat16, isOutput=False)
    cos_d = nc.declare_dram_parameter("cosT", [128, s], dt.bfloat16, isOutput=False)
    sin_d = nc.declare_dram_parameter("sinT", [128, s], dt.bfloat16, isOutput=False)
    msk_d = nc.declare_dram_parameter("masks", [128, n_mask * QC], dt.bfloat16, isOutput=False)
    eye_d = nc.declare_dram_parameter("eye", [128, 128], dt.bfloat16, isOutput=False)
    on128_d = nc.declare_dram_parameter("on128", [128, 1], dt.bfloat16, isOutput=False)
    on1_d = nc.declare_dram_parameter("on1", [1, 128], dt.bfloat16, isOutput=False)
    out_d = nc.declare_dram_parameter("out", [s, hid], dt.bfloat16, isOutput=True)

    with ExitStack() as ctx:
        tc = ctx.enter_context(tile.TileContext(nc))
        const = ctx.enter_context(tc.tile_pool(name="const", bufs=1))
        hpool = ctx.enter_context(tc.tile_pool(name="hpool", bufs=3))
        epool = ctx.enter_context(tc.tile_pool(name="epool", bufs=8))
        spool = ctx.enter_context(tc.tile_pool(name="spool", bufs=2))
        ppsum = ctx.enter_context(tc.tile_pool(name="ppsum", bufs=1, space="PSUM"))
        spsum = ctx.enter_context(tc.tile_pool(name="spsum", bufs=1, space="PSUM"))
        apsum = ctx.enter_context(tc.tile_pool(name="apsum", bufs=1, space="PSUM"))
        dpsum = ctx.enter_context(tc.tile_pool(name="dpsum", bufs=1, space="PSUM"))
        opsum = ctx.enter_context(tc.tile_pool(name="opsum", bufs=2, space="PSUM"))

        # ---- weights resident in SBUF; piece 0 gates the first matmuls ----
        NP = 8
        PT = HT // NP
        wq_sb = const.tile([128, HT * HQ * D], dt.bfloat16, tag="wq")
        wk_sb = const.tile([128, HT * D], dt.bfloat16, tag="wk")
        wv_sb = const.tile([128, HT * D], dt.bfloat16, tag="wv")
        wo_sb = const.tile([128, HQ * hid], dt.bfloat16, tag="wo")
        msk_sb = const.tile([128, n_mask * QC], dt.bfloat16, tag="msk")

        def load_w_range(t0, t1):
            a, b = t0 * HQ * D, t1 * HQ * D
            nc.sync.dma_start(wq_sb[:, a:b], wq_d[:, a:b])
            a, b = t0 * D, t1 * D
            nc.sync.dma_start(wk_sb[:, a:b], wk_d[:, a:b])
            nc.sync.dma_start(wv_sb[:, a:b], wv_d[:, a:b])

        def load_w_piece(p):
            load_w_range(p * PT, (p + 1) * PT)

        # inputs are demand-paged on first touch: gate the first matmul on a
        # single hid-tile of weights + activations, not a whole piece
        load_w_range(0, 1)
        # tiny consts off the sync queue so they never delay the hT stream
        eye_sb = const.tile([128, 128], dt.bfloat16, tag="eye")
        nc.scalar.dma_start(eye_sb[:], eye_d[:])
        on128_sb = const.tile([128, 1], dt.bfloat16, tag="on128")
        nc.scalar.dma_start(on128_sb[:], on128_d[:])
        on1_sb = const.tile([1, 128], dt.bfloat16, tag="on1")
        nc.scalar.dma_start(on1_sb[:], on1_d[:])

        # persistent per-chunk tensors
        q_sb = [[const.tile([128, QC], dt.bfloat16, tag=f"q{c}_{h}", name=f"q{c}_{h}")
                 for h in range(HQ)] for c in range(C)]
        kt_sb = [const.tile([128, QC], dt.bfloat16, tag=f"kt{c}", name=f"kt{c}")
                 for c in range(C)]
        v_sb = [[const.tile([128, 128], dt.bfloat16, tag=f"v{c}_{j}", name=f"v{c}_{j}")
                 for j in range(QC // 128)] for c in range(C)]
        at_sb = [[const.tile([128, QC], dt.bfloat16, tag=f"at{c}_{h}", name=f"at{c}_{h}")
                  for h in range(HQ)] for c in range(C)]

        cos_cur = [None]
        sin_cur = [None]
        acc_cur = {}             # slot -> psum accumulator
        # chunk 0 runs a merged single-pass projection using all 6 banks
        # (the attention-stream banks are idle until cs=1); chunks 1..3 run
        # two groups of 3 so sc/at/den stay free for the attention stream.
        P6 = [(ppsum, "p0"), (ppsum, "p1"), (ppsum, "p2"),
              (spsum, "sc"), (apsum, "at"), (dpsum, "den")]

        def drain_copy(eng_idx, dst, src):
            # gpsimd cannot read PSUM; alternate the two engines that can
            if eng_idx % 2 == 0:
                nc.scalar.copy(dst, src)
            else:
                nc.vector.tensor_copy(dst, src)

        # ---- projection: one tb unit = 4 hid tiles x 3 (or 6) accumulators ----
        def proj_tb(c, grp, tb):
            def run():
                nslots = 6 if grp is None else 3
                if (grp is None or grp == 0) and tb == 0:
                    cos_cur[0] = spool.tile([128, QC], dt.bfloat16, tag="cosc", name="cosc")
                    sin_cur[0] = spool.tile([128, QC], dt.bfloat16, tag="sinc", name="sinc")
                if tb == 0:
                    for slot in range(nslots):
                        pool, tag = P6[slot]
                        acc_cur[slot] = pool.tile([128, QC], dt.float32,
                                                  tag=tag, name=tag)
                htb = hpool.tile([128, HB * QC], dt.bfloat16, tag="htb", name="htb")
                base = (c * HT + tb * HB) * QC
                if c == 0 and tb == 0:
                    # fine-grained first batch: each hid-tile's activations and
                    # weights page in independently so t=0 starts ~4x earlier
                    for ts_ in range(HB):
                        nc.sync.dma_start(htb[:, ts_ * QC:(ts_ + 1) * QC],
                                          hT_d[:, base + ts_ * QC:base + (ts_ + 1) * QC])
                        if ts_ < HB - 1:
                            load_w_range(ts_ + 1, ts_ + 2)
                else:
                    nc.sync.dma_start(htb[:], hT_d[:, base:base + HB * QC])
                if (grp is None or grp == 0) and tb == 0:
                    nc.sync.dma_start(cos_cur[0][:], cos_d[:, c * QC:(c + 1) * QC])
                    nc.sync.dma_start(sin_cur[0][:], sin_d[:, c * QC:(c + 1) * QC])
                if c == 0 and tb < NP - 1:
                    load_w_piece(tb + 1)
                for ts_ in range(HB):
                    t = tb * HB + ts_
                    ht = htb[:, ts_ * QC:(ts_ + 1) * QC]
                    st, sp = (t == 0), (t == HT - 1)
                    for slot in range(nslots):
                        if grp is None:
                            if slot < 4:
                                w = wq_sb[:, t * HQ * D + slot * D: t * HQ * D + (slot + 1) * D]
                            elif slot == 4:
                                w = wk_sb[:, t * D:(t + 1) * D]
                            else:
                                w = wv_sb[:, t * D:(t + 1) * D]
                        elif slot < 2:
                            h = grp * 2 + slot
                            w = wq_sb[:, t * HQ * D + h * D: t * HQ * D + (h + 1) * D]
                        elif grp == 0:
                            w = wk_sb[:, t * D:(t + 1) * D]
                        else:
                            w = wv_sb[:, t * D:(t + 1) * D]
                        nc.tensor.matmul(acc_cur[slot][:], w, ht, start=st, stop=sp)
            return run

        # ---- rope on a drained (bf16) copy of a projection accumulator ----
        def rope_stream(a_ps, dest, eng_idx):
            a = spool.tile([128, QC], dt.bfloat16, tag="ropea", name="ropea")
            drain_copy(eng_idx, a[:], a_ps[:])
            b = spool.tile([128, QC], dt.bfloat16, tag="ropeb", name="ropeb")
            nc.gpsimd.dma_start(b[0:64, :], a[64:128, :])
            nc.gpsimd.dma_start(b[64:128, :], a[0:64, :])
            t1 = spool.tile([128, QC], dt.bfloat16, tag="ropet", name="ropet")
            nc.vector.tensor_mul(t1[:], a[:], cos_cur[0][:])
            nc.vector.tensor_mul(b[:], b[:], sin_cur[0][:])
            nc.vector.tensor_add(dest[:], t1[:], b[:])

        def emit_vt(c, vt_ps, eng_idx):
            vtT = spool.tile([128, QC], dt.bfloat16, tag="vtT", name="vtT")
            drain_copy(eng_idx, vtT[:], vt_ps[:])
            for j in range(QC // 128):
                tp = opsum.tile([128, 128], dt.bfloat16, tag="ops", name="tp")
                nc.tensor.transpose(tp[:], vtT[:, j * 128:(j + 1) * 128], eye_sb[:])
                nc.vector.tensor_copy(v_sb[c][j][:], tp[:])

        def proj_boundary(c, grp):
            # drain in the order the next units reuse the banks: p0 first
            def run():
                if grp is None:
                    for h in range(4):
                        rope_stream(acc_cur[h], q_sb[c][h][:], h)
                    rope_stream(acc_cur[4], kt_sb[c][:], 0)
                    emit_vt(c, acc_cur[5], 1)
                else:
                    rope_stream(acc_cur[0], q_sb[c][grp * 2][:], 0)
                    rope_stream(acc_cur[1], q_sb[c][grp * 2 + 1][:], 1)
                    if grp == 0:
                        rope_stream(acc_cur[2], kt_sb[c][:], 0)
                    else:
                        emit_vt(c, acc_cur[2], 0)
            return run

        # ---- attention stream items for chunk c (uses blocks[c]) ----
        att_state = {}

        def make_att_items(c):
            items = []
            blks = blocks[c]
            n = len(blks)

            def mk_sc(h, i):
                def run():
                    g, mid = blks[i]
                    kc, j = g // (QC // 128), g % (QC // 128)
                    sc = spsum.tile([128, QC], dt.float32, tag="sc", name="sc")
                    nc.tensor.matmul(sc[:], kt_sb[kc][:, j * 128:(j + 1) * 128],
                                     q_sb[c][h][:], start=True, stop=True)
                    e = epool.tile([128, QC], dt.bfloat16, tag="e", name="e")
                    nc.scalar.activation(e[:], sc[:], mybir.ActivationFunctionType.Exp,
                                         scale=float(INV_NORM))
                    if mid is not None:
                        nc.vector.tensor_mul(e[:], e[:],
                                             msk_sb[:, mid * QC:(mid + 1) * QC])
                    if i == 0:
                        esum = spool.tile([128, QC], dt.float32, tag="esum", name="esum")
                        nc.vector.tensor_copy(esum[:], e[:])
                        att_state['esum'] = esum
                    elif i < n - 1:
                        eng = nc.vector if i % 2 else nc.gpsimd
                        eng.tensor_add(att_state['esum'][:], att_state['esum'][:], e[:])
                    else:
                        # final add rounds the fp32 running sum to bf16 so the
                        # denominator matmul runs at full bf16 PE rate
                        esb = spool.tile([128, QC], dt.bfloat16, tag="esumb", name="esb")
                        eng = nc.vector if i % 2 else nc.gpsimd
                        eng.tensor_add(esb[:], att_state['esum'][:], e[:])
                        att_state['esumb'] = esb
                    att_state[('e', h, i)] = e
                return run

            def mk_av(h, i):
                def run():
                    g, _ = blks[i]
                    kc, j = g // (QC // 128), g % (QC // 128)
                    if i == 0:
                        att_state['at'] = apsum.tile([128, QC], dt.float32,
                                                     tag="at", name="at")
                    e = att_state.pop(('e', h, i))
                    nc.tensor.matmul(att_state['at'][:], v_sb[kc][j][:], e[:],
                                     start=(i == 0), stop=(i == n - 1))
                    if i == n - 1:
                        atu = spool.tile([128, QC], dt.bfloat16, tag="atu", name="atu")
                        nc.scalar.copy(atu[:], att_state['at'][:])
                        att_state['atu'] = atu
                return run

            def mk_den(h):
                def run():
                    den = dpsum.tile([1, QC], dt.float32, tag="den", name="den")
                    nc.tensor.matmul(den[:], on128_sb[:], att_state['esumb'][:],
                                     start=True, stop=True)
                    recf = spool.tile([1, QC], dt.float32, tag="recf", name="recf")
                    nc.vector.reciprocal_approx_fast(recf[:], den[:])
                    rcb = spool.tile([1, QC], dt.bfloat16, tag="recfb", name="rcb")
                    nc.scalar.copy(rcb[:], recf[:])
                    att_state['recf'] = rcb
                return run

            def mk_bn(h):
                def run():
                    bc = opsum.tile([128, QC], dt.float32, tag="ops", name="bc")
                    nc.tensor.matmul(bc[:], on1_sb[:], att_state['recf'][:],
                                     start=True, stop=True)
                    nc.vector.tensor_mul(at_sb[c][h][:], att_state['atu'][:], bc[:])
                return run

            for h in range(HQ):
                for i in range(min(LOOK, n)):
                    items.append(mk_sc(h, i))
                for i in range(n):
                    items.append(mk_av(h, i))
                    if i + LOOK < n:
                        items.append(mk_sc(h, i + LOOK))
                items.append(mk_den(h))
                items.append(mk_bn(h))
            return items

        # ---- output projection items for chunk c: 32 groups ----
        # chunks >= 2 run at the tail, when the projection banks are free:
        # rotate over 5 banks and alternate drain engines so the PE never
        # waits on a PSUM drain round-trip.
        OUT_ROT = [(opsum, "ops"), (ppsum, "p0"), (opsum, "ops"),
                   (ppsum, "p1"), (ppsum, "p2")]

        def make_out_items(c):
            items = []

            def mk_grp(oc, r, idx):
                def run():
                    if c >= 2:
                        pool, tag = OUT_ROT[idx % 5]
                    else:
                        pool, tag = opsum, "ops"
                    o_ps = pool.tile([128, QC], dt.float32, tag=tag, name="o_ps")
                    for h in range(HQ):
                        nc.tensor.matmul(
                            o_ps[:], at_sb[c][h][:, r * 128:(r + 1) * 128],
                            wo_sb[:, h * hid + oc * QC: h * hid + (oc + 1) * QC],
                            start=(h == 0), stop=(h == HQ - 1))
                    ob = spool.tile([128, QC], dt.bfloat16, tag="ob", name="ob",
                                    bufs=3)
                    drain_copy(idx % 2, ob[:], o_ps[:])
                    row = c * QC + r * 128
                    nc.sync.dma_start(out_d[row:row + 128, oc * QC:(oc + 1) * QC], ob[:])
                return run

            idx = 0
            for oc in range(hid // QC):
                for r in range(QC // 128):
                    items.append(mk_grp(oc, r, idx))
                    idx += 1
            return items

        # ---- paced interleave of the three streams per chunk slot ----
        def cdiv(a, b):
            return -(-a // b)

        for cs in range(C + 2):
            units = []
            if cs == 0:
                for tb in range(HT // HB):
                    units.append(proj_tb(0, None, tb))
                units.append(proj_boundary(0, None))
            elif cs < C:
                for grp in range(2):
                    for tb in range(HT // HB):
                        units.append(proj_tb(cs, grp, tb))
                    units.append(proj_boundary(cs, grp))
            else:
                units = [None] * 8
            att_items = make_att_items(cs - 1) if 1 <= cs <= C else []
            out_items = make_out_items(cs - 2) if cs >= 2 else []
            if cs == 1:
                # off the sync queue so they never delay the hT stream
                nc.gpsimd.dma_start(msk_sb[:], msk_d[:])
                nc.gpsimd.dma_start(wo_sb[:], wo_d[:])
            ai = oi = 0
            nu = len(units)
            # attention items start two units late so the previous chunk's
            # rope chains complete before its first score hits the PE head
            DLY = 2 if cs < C else 1
            for ui, u in enumerate(units):
                if u is not None:
                    u()
                at_t = cdiv(len(att_items) * max(0, ui + 1 - DLY), nu - DLY)
                ot_t = cdiv(len(out_items) * (ui + 1), nu)
                while ai < at_t or oi < ot_t:
                    if oi < ot_t and cs >= C:
                        out_items[oi]()
                        oi += 1
                    if ai < at_t:
                        att_items[ai]()
                        ai += 1
                    if ai < at_t:
                        att_items[ai]()
                        ai += 1
                    if oi < ot_t and cs < C:
                        out_items[oi]()
                        oi += 1

    nc.compile()
    return nc


def _prep_inputs(hidden_states, attention_mask, Wq, Wk, Wv, Wo):
    """Host-side sharding + layout prep. Returns (in_maps, blocks, n_mask, s, hid)."""
    hs = np.asarray(hidden_states)
    assert hs.shape[0] == 1, "kernel assumes batch 1"
    s, hid = hs.shape[1], hs.shape[2]
    mask = np.asarray(attention_mask)[0]
    Wq = np.asarray(Wq); Wk = np.asarray(Wk); Wv = np.asarray(Wv); Wo = np.asarray(Wo)

    # SBUF-image packing: x[(t p), c] -> [p, (t c)] so DMAs are contiguous
    def pack(w, tiles):
        return np.ascontiguousarray(
            w.reshape(tiles, 128, -1).transpose(1, 0, 2).reshape(128, -1)
        ).astype(BF16)

    hTn = np.asarray(hs[0].T).reshape(hid // 128, 128, s // QC, QC)
    hT = np.ascontiguousarray(hTn.transpose(1, 2, 0, 3).reshape(128, -1)).astype(BF16)
    # layout: hT[p, ((c * HT + t) * QC + q)]
    cosT, sinT = _rope_tables(s)
    blocks, mask_tiles = _classify_mask(mask, s)
    masks_bf = mask_tiles.astype(BF16)
    eye = np.eye(128, dtype=np.float32).astype(BF16)
    on128 = np.ones((128, 1), dtype=np.float32).astype(BF16)
    on1 = np.ones((1, 128), dtype=np.float32).astype(BF16)

    n_mask = masks_bf.shape[0]
    masks_pk = np.ascontiguousarray(
        masks_bf.transpose(1, 0, 2).reshape(128, n_mask * QC))

    in_maps = []
    for i in range(NCORES):
        wq_i = pack(Wq[:, i * HQ:(i + 1) * HQ, :].reshape(hid, HQ * D), hid // 128)
        wk_i = pack(Wk[:, i, :], hid // 128)
        wv_i = pack(Wv[:, i, :], hid // 128)
        wo_i = pack(Wo[i * HQ:(i + 1) * HQ].reshape(HQ * D, hid), HQ)
        in_maps.append({
            "hT": hT, "wq": wq_i, "wk": wk_i, "wv": wv_i, "wo": wo_i,
            "cosT": cosT, "sinT": sinT, "masks": masks_pk, "eye": eye,
            "on128": on128, "on1": on1,
        })
    return in_maps, blocks, n_mask, s, hid


def _run(hidden_states, attention_mask, Wq, Wk, Wv, Wo, trace=False):
    from concourse.bass_utils import run_bass_kernel_spmd

    in_maps, blocks, n_mask, s, hid = _prep_inputs(
        hidden_states, attention_mask, Wq, Wk, Wv, Wo)
    nc = _build_program(s, hid, blocks, n_mask)
    res = run_bass_kernel_spmd(nc, in_maps, core_ids=list(range(NCORES)),
                               trace=trace)
    out = np.zeros((s, hid), np.float32)
    for i in range(NCORES):
        out += res.results[i]["out"].astype(np.float32)
    return out[None, :, :], res


def kernel(hidden_states, attention_mask, Wq, Wk, Wv, Wo):
    out, _ = _run(hidden_states, attention_mask, Wq, Wk, Wv, Wo, trace=False)
    return out
